# revision 16
# baseline (speedup 1.0000x reference)
"""MoE layer (T=4096, H=1024, F=2048, E=8, top-2) on 8 Trainium2 NeuronCores.

Strategy (expert-parallel, per the sharding hint):
  - Router runs on host (67 MFLOP, 0.02% of total work) to produce the
    token->expert dispatch; this implements the "all-to-all dispatch by
    routed expert" as host-side sharding, which is where sharding lives in
    this harness's contract (full inputs in, full output out).
  - Core e holds expert e's weights (w1[e], w2[e]) and processes only the
    tokens routed to it (capacity-padded to the max expert load).
  - Device per core: hT = w1[e]^T-oriented matmul producing g^T = gelu(x@w1)
    in [F, C] layout, then y^T = w2^T-stationary matmuls over g^T -- no
    on-device transposes needed; prob scaling happens in the host combine.
  - Host scatter-adds each expert's [count, H] slice into the [T, H] output.

Matmuls run in bf16 with fp32 PSUM accumulation (rel err ~3e-3 vs the fp32
reference on this input set; measured host-side before committing to it).
"""

import numpy as np
import ml_dtypes

T, H, F, E, TOPK = 4096, 1024, 2048, 8, 2

_BUILT = {}  # cache: (C, maxc, dtype_tag, gelu_tag) -> bass.Bass


def _route(hidden_states, router_weight):
    """Host router: returns (token_idx[e], prob[e]) per expert."""
    logits = hidden_states.astype(np.float32) @ router_weight.astype(np.float32)
    # top-2 by value (descending); ties broken like jax.lax.top_k (lowest index)
    order = np.argsort(-logits, axis=1, kind="stable")
    top_idx = order[:, :TOPK]                                   # [T, 2]
    top_vals = np.take_along_axis(logits, top_idx, axis=1)      # [T, 2]
    m = top_vals.max(axis=1, keepdims=True)
    ex = np.exp(top_vals - m)
    top_probs = ex / ex.sum(axis=1, keepdims=True)              # [T, 2]
    tok_per_e, prob_per_e = [], []
    for e in range(E):
        mask = top_idx == e                                     # [T, 2]
        tok = np.nonzero(mask.any(axis=1))[0]
        p = (top_probs * mask).sum(axis=1)[tok].astype(np.float32)
        tok_per_e.append(tok)
        prob_per_e.append(p)
    return tok_per_e, prob_per_e


def _strip_redundant_same_engine_waits(nc, margin=8):
    """Drop sem waits that FIFO program order on the same engine already
    guarantees (cumulative same-engine updates >= wait_value + margin).

    Tile occasionally synthesizes such waits (e.g. on the first phase-B copy);
    they are semantically redundant but overflow the 1-wait sync budget of the
    AC/TS instruction structs in walrus codegen. margin covers the engine
    queue depth so even in-flight-but-unretired updates can't be relied on.
    """
    from collections import defaultdict
    for f in nc.m.functions:
        counts = defaultdict(int)  # sem_id -> updates so far (program order)
        for b in f.blocks:
            for i in b.instructions:
                si = i.sync_info
                if si is None:
                    continue
                ups = [u for u in (si.on_update or [])
                       if u.sync_type == "semaphore"
                       and u.update_mode in ("sem-inc", "sem-add-imm")]
                own_sems = {u.id for u in ups}
                ws = list(si.on_wait or [])
                keep = []
                for w in ws:
                    if (w.sync_type == "semaphore"
                            and w.wait_mode == "sem-ge-imm"
                            and w.id in own_sems):
                        # wait on the stream this instruction itself belongs
                        # to (it updates the same sem): FIFO order within the
                        # stream makes it redundant once enough prior updates
                        # exist. DMA queue sems bump by 16 per transfer, so
                        # margin only applies to 1-inc engine sems.
                        m = margin if all(
                            u.update_value == 1 for u in ups if u.id == w.id
                        ) else 0
                        if counts[w.id] >= w.wait_value + m:
                            continue
                    keep.append(w)
                if len(keep) != len(ws):
                    si.on_wait = keep
                    i.sync_info = si
                for u in ups:
                    counts[u.id] += u.update_value


def _split_overloaded_waits(nc, max_waits=1):
    """The TPB instruction structs encode at most one sem-ge wait (plus
    updates); walrus errors with "Too many sync wait commands" beyond that.
    For any instruction still carrying several sem-ge waits after the
    redundancy strip, move the excess onto NOPs inserted just before it on
    the same engine -- a strictly more conservative ordering."""
    import concourse.mybir as mybir
    import bass_rust
    for f in nc.m.functions:
        for b in f.blocks:
            il = b.instructions
            idx = 0
            while idx < len(il):
                i = il[idx]
                si = i.sync_info
                if type(i).__name__ == "InstEventSemaphore" or si is None:
                    idx += 1
                    continue
                sem_ws = [w for w in (si.on_wait or [])
                          if w.sync_type == "semaphore"
                          and w.wait_mode == "sem-ge-imm"]
                other_ws = [w for w in (si.on_wait or []) if w not in sem_ws]
                if len(sem_ws) > max_waits:
                    si.on_wait = other_ws + sem_ws[-max_waits:]
                    i.sync_info = si
                    rest = sem_ws[:-max_waits]
                    pos = idx
                    for j in range(0, len(rest), max_waits):
                        n = mybir.InstNoOp(
                            name=nc.get_next_instruction_name(),
                            ins=[], outs=[])
                        n.engine = i.engine
                        n.sync_info = bass_rust.SyncInfo(
                            on_wait=rest[j:j + max_waits], on_update=[])
                        il.insert(pos, n)
                        pos += 1
                        idx += 1
                idx += 1


def _trim_redundant_barriers(nc):
    """Remove the Tile entry barrier from the main block (delays the first
    DMA trigger ~0.7us; body data deps are fully covered by per-tile sems --
    the only thing it orders is const-tile memsets, which finish ~8us before
    their first reader) and the second end-block barrier round (the runtime
    epilogue's own butterfly follows immediately)."""
    bar_ids = {int(k) for k, v in nc.m.ant_sem_names.items()
               if any("barrier_" in n for n in v)}

    def is_bar(i):
        si = i.sync_info
        if si is None or type(i).__name__ not in ("InstDrain",
                                                  "InstEventSemaphore"):
            return False
        ids = [w.id for w in (si.on_wait or [])] +               [u.id for u in (si.on_update or [])]
        return bool(ids) and all(x in bar_ids for x in ids)

    f = nc.m.functions[0]
    main = f.blocks[0]
    main.instructions[:] = [i for i in main.instructions if not is_bar(i)]
    endb = f.blocks[-1]
    il = endb.instructions
    isa = [ix for ix, i in enumerate(il) if type(i).__name__ == "InstISA"]
    if isa:
        cut = isa[-1] + 1
        il[:] = il[:cut] + [i for i in il[cut:] if not is_bar(i)]


def _build(C, dtype_tag="bf16", gelu_tag="tanh", valid_c=None, warm_mms=8,
           x_split=4, tail_split=True):
    """Build the per-core Bass program. C = token capacity (multiple of 128);
    valid_c = number of real (non-pad) token columns (phase A only computes
    g^T up to valid_c rounded to 64; the tail is zero-filled)."""
    import concourse.bass as bass
    import concourse.mybir as mybir
    import concourse.tile as tile

    DT = {"bf16": mybir.dt.bfloat16, "f32": mybir.dt.float32}[dtype_tag]
    F32 = mybir.dt.float32
    GELU = {
        "tanh": mybir.ActivationFunctionType.Gelu_apprx_tanh,
        "erf": mybir.ActivationFunctionType.Gelu,
    }[gelu_tag]
    mm_cast = (lambda ap: ap.bitcast(mybir.dt.float32r)) if dtype_tag == "f32" \
        else (lambda ap: ap)

    KH, KF = H // 128, F // 128        # 8 h-tiles, 16 f-tiles
    CB = C // 128                      # token 128-blocks
    NH = H // 512                      # 2 output column chunks
    # phase A only computes g^T columns for real tokens (rounded up to 64);
    # the remaining pad columns are zero-filled so phase B's lhsT reads are
    # defined. y rows beyond the real count are garbage the host ignores.
    CA = C if valid_c is None else min(C, -(-valid_c // 64) * 64)
    chunks = [(c0, min(512, CA - c0)) for c0 in range(0, CA, 512)]

    nc = bass.Bass()
    xT_d = nc.dram_tensor("xT", [H, C], DT, kind="ExternalInput")
    # w1 arrives pre-tiled: w1t[fm, p, hk*128+c] = w1[hk*128+p, fm*128+c], so
    # each [128, 128] lhsT tile is a contiguous free-dim slice and one fm's
    # worth (0.5 MB) is a single DMA -- keeps the first matmul group's DMA
    # prefix tiny instead of needing all of w1.
    w1_d = nc.dram_tensor("w1t", [KF, 128, KH * 128], DT, kind="ExternalInput")
    w2_d = nc.dram_tensor("w2", [F, H], DT, kind="ExternalInput")
    # y is produced transposed ([H, C]): phase B keeps tokens on the moving
    # axis so the pad columns beyond CA are never computed. Host transposes.
    # bf16 output: |y| <= ~2 so the 0.4% rounding is far inside the error
    # budget, and halving the writeback drains the end-of-kernel DMA backlog
    # sooner.
    y_d = nc.dram_tensor("y", [H, C], mybir.dt.bfloat16, kind="ExternalOutput")

    with tile.TileContext(nc) as tc:
        with tc.tile_pool(name="persist", bufs=1) as wp, \
             tc.tile_pool(name="psumA", bufs=5, space="PSUM") as pp_a, \
             tc.tile_pool(name="psumB", bufs=3, space="PSUM") as pp_b, \
             tc.tile_pool(name="outs", bufs=4) as op:

            xT_s = [wp.tile([128, C], DT, name=f"xT{k}") for k in range(KH)]
            w1_s = [wp.tile([128, KH * 128], DT, name=f"w1t{k}")
                    for k in range(KF)]
            w2_s = [wp.tile([128, H], DT, name=f"w2{k}") for k in range(KF)]

            # DMA issue order tracks first use. Phase A runs chunk-outer, so
            # the critical prefix is w1[0] (256 KB, the first group's weights)
            # + xT chunk0 (1 MB); later groups only need one 256 KB w1 tile
            # each ~2us, which SP's ~650ns/issue rate supplies easily.
            # Only SP + ACT can push HWDGE; ACT takes 4 early transfers and
            # must be free for gelus by ~15us.
            def xchunk(k, c0, cw, e):
                e.dma_start(xT_s[k][:, c0:c0 + cw],
                            xT_d[k * 128:(k + 1) * 128, c0:c0 + cw])

            # Critical-prefix DMAs only: xT chunk0 + w1[0..2] + prob. All
            # other input DMAs are paced behind phase-A progress (below) so
            # they don't steal aggregate HBM bandwidth from this prefix.
            (c0a, cwa) = chunks[0]
            # w1[0] gates the very first matmul; one 256KB transfer on a
            # single queue takes ~2.9us, so split it across two queues
            # (subtile deps let MM(hk) start once its half has landed).
            half = KH * 128 // 2
            nc.scalar.dma_start(w1_s[0][:, :half], w1_d[0][:, :half])
            nc.sync.dma_start(w1_s[0][:, half:], w1_d[0][:, half:])
            for k in range(x_split):
                xchunk(k, c0a, cwa, nc.scalar)
            nc.scalar.dma_start(w1_s[1][:], w1_d[1])
            for k in range(x_split, KH):
                xchunk(k, c0a, cwa, nc.sync)
            nc.sync.dma_start(w1_s[2][:], w1_d[2])

            # PE warm-up: 8 dummy matmuls on zeroed tiles while the critical
            # DMAs land. HAM flips the PE clock 1.2->2.4 GHz after ~3.4us of
            # sustained activity; burning the cold window on throwaway work
            # makes the real matmuls start warm. GpSimd memsets start
            # earliest (no DMA deps).
            if warm_mms:
                dmy_l = wp.tile([128, 128], DT, name="dmy_l")
                dmy_r = wp.tile([128, 512], DT, name="dmy_r")
                nc.gpsimd.memset(dmy_l[:], 0)
                nc.gpsimd.memset(dmy_r[:], 0)
                ps_w = pp_b.tile([128, 512], F32, name="psW", tag="psB")
                for r in range(warm_mms):
                    nc.tensor.matmul(ps_w[:], mm_cast(dmy_l[:]),
                                     mm_cast(dmy_r[:]),
                                     start=(r == 0), stop=(r == warm_mms - 1))
            gT_s = [wp.tile([128, C], DT, name=f"gT{k}") for k in range(KF)]
            if CA < C:
                for k in range(KF):
                    nc.vector.memset(gT_s[k][:, CA:C], 0)

            # Non-critical input DMAs, paced behind phase-A groups so the
            # transfers trail compute instead of competing with the critical
            # prefix for aggregate HBM bandwidth. pace_plan[g] = DMAs to
            # issue once group g's matmuls retire (~1.7us per warm group).
            pace_plan = {}

            def plan(g, dst, src):
                pace_plan.setdefault(g, []).append((dst, src))

            for k in range(3, KF):
                plan(max(0, k - 6), w1_s[k][:], w1_d[k])   # needed at group k
            for ci, (c0, cw) in enumerate(chunks[1:]):     # needed at chunk 1+
                for k in range(KH):
                    plan(4 + 5 * ci + k // 2, xT_s[k][:, c0:c0 + cw],
                         xT_d[k * 128:(k + 1) * 128, c0:c0 + cw])
            for k in range(KF):                            # needed in phase B
                plan(10 + k, w2_s[k][:], w2_d[k * 128:(k + 1) * 128, :])

            from concourse.tile_rust import add_dep_helper

            # Phase A: g^T[f, c] = gelu(sum_h w1[h, f] * x[c, h]), [F, C]
            # layout. chunk-outer: early groups reuse xT chunk0 and need just
            # one fresh w1 tile each, keeping the DMA critical path short.
            gi = 0
            for (c0, cw) in chunks:
                for fm in range(KF):
                    ps = pp_a.tile([128, 512], F32, name="psA", tag="psA")
                    for hk in range(KH):
                        mm = nc.tensor.matmul(
                            ps[:, :cw],
                            mm_cast(w1_s[fm][:, hk * 128:(hk + 1) * 128]),
                            mm_cast(xT_s[hk][:, c0:c0 + cw]),
                            start=(hk == 0),
                            stop=(hk == KH - 1),
                        )
                    nc.scalar.activation(gT_s[fm][:, c0:c0 + cw], ps[:, :cw], GELU)
                    for (dst, src) in pace_plan.get(gi, []):
                        d = nc.sync.dma_start(dst, src)
                        add_dep_helper(d.ins, mm.ins, sync=True,
                                       reason="dma pacing")
                    gi += 1

            # Scheduler fence between phases: without it Tile reorders the ACT
            # stream and synthesizes same-engine waits on the phase-B copies,
            # overflowing the AC struct's 1-wait budget.
            tc.no_sync_barrier()

            # Phase B: y^T[h, c] = sum_f w2[f, h] * g^T[f, c].
            # w2 is the stationary operand and tokens the moving axis, so the
            # pad columns beyond CA cost nothing. Routing-prob scaling happens
            # on the host during scatter-add (it is per token = per column
            # here, which ACT cannot broadcast).
            for hm in range(KH):
                for ci, (c0, cw) in enumerate(chunks):
                    ps = pp_b.tile([128, 512], F32, name="psB", tag="psB")
                    for fk in range(KF):
                        nc.tensor.matmul(
                            ps[:, :cw],
                            mm_cast(w2_s[fk][:, hm * 128:(hm + 1) * 128]),
                            mm_cast(gT_s[fk][:, c0:c0 + cw]),
                            start=(fk == 0),
                            stop=(fk == KF - 1),
                        )
                    # unique slot per output tile: a reused slot would add a
                    # WAR wait on the DMA-out to the copy instruction.
                    yt = op.tile([128, 512], mybir.dt.bfloat16, name="yt",
                                 tag="yt", bufs=KH * len(chunks))
                    nc.scalar.activation(
                        yt[:, :cw], ps[:, :cw],
                        mybir.ActivationFunctionType.Copy)
                    # final hm: issue the out-DMAs from ACT (right after its
                    # copies, in parallel with SP draining earlier queues) --
                    # trims the serial copy->issue->transfer endgame.
                    # tail_split: spread the last hm's issues across SP+ACT
                    # so they go out concurrently instead of serially on ACT.
                    if hm == KH - 1:
                        deng = (nc.sync if tail_split
                                and ci < len(chunks) - 1 else nc.scalar)
                    else:
                        deng = nc.sync
                    deng.dma_start(
                        y_d[hm * 128:(hm + 1) * 128, c0:c0 + cw],
                        yt[:, :cw])

    _strip_redundant_same_engine_waits(nc)
    _split_overloaded_waits(nc)
    _trim_redundant_barriers(nc)
    return nc


def _make_in_maps(hidden_states, w1, w2, tok_per_e, prob_per_e, C,
                  dtype_tag="bf16"):
    np_dt = {"bf16": ml_dtypes.bfloat16, "f32": np.float32}[dtype_tag]
    in_maps = []
    for e in range(E):
        tok = tok_per_e[e]
        xg = np.zeros((H, C), dtype=np_dt)
        xg[:, :len(tok)] = hidden_states[tok].T.astype(np_dt)
        KH, KF = H // 128, F // 128
        w1t = (w1[e].astype(np_dt).reshape(KH, 128, KF, 128)
               .transpose(2, 1, 0, 3).reshape(KF, 128, KH * 128))
        m = {
            "xT": np.ascontiguousarray(xg),
            "w1t": np.ascontiguousarray(w1t),
            "w2": np.ascontiguousarray(w2[e].astype(np_dt)),
        }
        in_maps.append(m)
    return in_maps


def _gelu_tanh(x):
    return 0.5 * x * (1.0 + np.tanh(0.7978845608028654
                                    * (x + 0.044715 * x * x * x)))


# Device capacity per expert. sum(counts) == E*1024 exactly (top-2 of 4096
# tokens over 8 experts), so max(count) >= 1024 always and capping at 1024
# makes the device schedule exactly two 512-wide chunks -- the narrow tail
# chunk (~13us of dispatch-floor-limited matmuls for <9% of the columns)
# disappears. The few overflow tokens of overloaded experts are computed on
# the host in fp32 (~1 GFLOP, ~30ms) and merged in the combine.
DEV_CAP = 1024


def kernel(hidden_states, router_weight, w1, w2):
    from concourse.bass_utils import run_bass_kernel_spmd

    hidden_states = np.asarray(hidden_states, dtype=np.float32)
    router_weight = np.asarray(router_weight, dtype=np.float32)
    w1 = np.asarray(w1, dtype=np.float32)
    w2 = np.asarray(w2, dtype=np.float32)

    tok_all, prob_all = _route(hidden_states, router_weight)
    tok_per_e = [t[:DEV_CAP] for t in tok_all]
    prob_per_e = [p[:DEV_CAP] for p in prob_all]
    maxc = max(len(t) for t in tok_per_e)
    C = max(128, -(-maxc // 128) * 128)          # capacity, multiple of 128
    CB = C // 128

    dtype_tag, gelu_tag = "bf16", "tanh"
    key = (C, maxc, dtype_tag, gelu_tag)
    if key not in _BUILT:
        _BUILT[key] = _build(C, dtype_tag, gelu_tag, valid_c=maxc)
    nc = _BUILT[key]

    in_maps = _make_in_maps(hidden_states, w1, w2, tok_per_e, prob_per_e, C,
                            dtype_tag)

    try:
        res = run_bass_kernel_spmd(nc, in_maps, core_ids=list(range(E)))
    except Exception:
        # transient device hiccups (NRT timeouts etc.) usually clear on retry
        import time as _time
        _time.sleep(2)
        res = run_bass_kernel_spmd(nc, in_maps, core_ids=list(range(E)))

    out = np.zeros((T, H), dtype=np.float32)
    for e in range(E):
        tok = tok_per_e[e]
        yT = res.results[e]["y"]          # [H, C] bf16, cols >= CA garbage
        out[tok] += prob_per_e[e][:, None] * \
            yT[:, :len(tok)].T.astype(np.float32)
        # host fp32 path for tokens beyond the device capacity
        ov = tok_all[e][DEV_CAP:]
        if len(ov):
            g = _gelu_tanh(hidden_states[ov] @ w1[e])
            out[ov] += prob_all[e][DEV_CAP:, None] * (g @ w2[e])
    return out

